# revision 1
# baseline (speedup 1.0000x reference)
"""ConvNetWordEncoder Trainium2 kernel.

Computes, for a batch of words (each a sequence of L=16 character ids):
  x = emb_table[words]                          # [L, N, D] character embeddings
  y = conv1d(x, conv_w, pad=1) + b              # [N, D, L]
  out = max_t relu(y)                           # [N, D]

Key algebraic rewrite: the embedding lookup is linear (x = E^T @ onehot), so
the conv collapses into per-tap fused tables M_k = E @ W_k^T of shape
[128 letters, 300 channels]:
  z_t[word, :] = sum_k M_k^T @ onehot(letter at t+k-1)
This shrinks the contraction from 900 (=D*K) to 128 (letters) per tap and
turns the embedding gather into the matmul itself.  relu/bias commute with
the temporal max, so bias+relu are applied once after the max.

Per core (data-parallel over N: 2048 words/core, 16 groups of 128 words):
  - a stride-0 DMA replicates each group's letter row across all 128
    partitions (no compute-engine cost),
  - GPSIMD builds the exact one-hot with a tensor_scalar is_equal against
    a per-partition iota,
  - PE conv: for each s, onehot_s is the stationary operand [128l, 128w],
    the three fused tables stream as moving operands [128l, 300c],
    accumulating z_{s-1}, z_s, z_{s+1} in PSUM (float32r, 1 cycle/row).
    z tiles are paired two-per-PSUM-tile (bank-aligned halves),
  - two independent DVE max chains (even/odd pairs, ACT-seeded) interleave
    so the serial-max semaphore latency hides; chains merge while the last
    matmuls run, so only one pair-max + one fused fold trail the PE,
  - the conv bias is folded into the middle-tap table on the host (tap k=1
    fires exactly once per t), and relu fuses into the final fold via
    relu(max(a,b)) == max(max(a,0),b) - one scalar_tensor_tensor.
Output orientation is [words, channels] directly - no transposes anywhere.

The first group's one-hot is built in four 512-column chunks so the first
matmuls gate on a quarter of the broadcast (cuts head latency).
"""

import numpy as np
from contextlib import ExitStack

import concourse.tile as tile
from concourse import bacc, mybir
from concourse.bass_utils import run_bass_kernel_spmd

HIDDEN = 300
NLET = 128
KSIZE = 3
L = 16
NWORDS = 16384
NCORES = 8
NW = NWORDS // NCORES          # 2048 words per core
GROUPS = NW // 128             # 16 groups of 128 words
FP32 = mybir.dt.float32
FP32R = mybir.dt.float32r

_cache = {}


def _build(iters=1, onehot_engine="pool"):
    key = (iters, onehot_engine)
    if key in _cache:
        return _cache[key]
    nc = bacc.Bacc("TRN2", target_bir_lowering=False, debug=False,
                   num_devices=NCORES)

    wordsg_d = nc.dram_tensor("wordsg", [GROUPS, L * 128], FP32,
                              kind="ExternalInput")
    wfuse_d = nc.dram_tensor("wfuse", [KSIZE, NLET, HIDDEN], FP32R,
                             kind="ExternalInput")
    iotaf_d = nc.dram_tensor("iotaf", [128, 1], FP32, kind="ExternalInput")
    out_d = nc.dram_tensor("out", [NW, HIDDEN], FP32, kind="ExternalOutput")

    Sq = mybir.ActivationFunctionType.Square
    Relu = mybir.ActivationFunctionType.Relu
    Max = mybir.AluOpType.max
    Add = mybir.AluOpType.add
    Eq = mybir.AluOpType.is_equal

    with tile.TileContext(nc) as tc, ExitStack() as ctx:
        const = ctx.enter_context(tc.tile_pool(name="const", bufs=1))
        bcpool = ctx.enter_context(tc.tile_pool(name="bc", bufs=8))
        ohpool = ctx.enter_context(tc.tile_pool(name="oh", bufs=8))
        rmpool = ctx.enter_context(tc.tile_pool(name="rm", bufs=6))
        outpool = ctx.enter_context(tc.tile_pool(name="outp", bufs=4))
        pz = ctx.enter_context(tc.tile_pool(name="pz", bufs=4, space="PSUM"))

        wfuse = const.tile([NLET, KSIZE, HIDDEN], FP32R)
        for k in range(KSIZE):
            nc.sync.dma_start(wfuse[:, k, :], wfuse_d.ap()[k])
        iota_t = const.tile([128, 1], FP32)
        nc.sync.dma_start(iota_t[:], iotaf_d.ap()[:])
        # PE clock pre-warm: the tensor engine ramps to full rate only
        # after ~3us of continuous activity.  The PE is otherwise idle
        # while the first one-hot chunk is built, so burn that window on
        # dummy matmuls (dependent only on the already-loaded weights) and
        # enter the real conv already warm.
        warm_ps = pz.tile([128, 2, 512], FP32, tag="zp", name="warm")
        for _ in range(30):
            nc.tensor.matmul(warm_ps[0:1, 0, 0:HIDDEN], wfuse[0:1, 0, 0:1],
                             wfuse[0:1, 0, :], start=True, stop=True)

        NP = L // 2  # 8 z-pairs per group
        for it in range(iters):
            for g in range(GROUPS):
                # letter row replicated across partitions by stride-0 DMA.
                # The first group of the kernel is split into 4 chunks so the
                # first matmuls only gate on 1/4 of the one-hot (head latency).
                first = True
                nchunk = 4
                ohc = []
                csz = (L * 128) // nchunk
                for q in range(nchunk):
                    bcast = bcpool.tile([128, csz], FP32, tag="bcast",
                                        name="bcast")
                    nc.sync.dma_start(
                        bcast[:],
                        wordsg_d.ap()[g, q * csz:(q + 1) * csz]
                        .partition_broadcast(128))
                    oh = ohpool.tile([128, csz], FP32R, tag="ohblk",
                                     name="ohc")
                    nc.gpsimd.tensor_scalar(oh[:], bcast[:], iota_t[:],
                                            None, Eq)
                    ohc.append(oh)

                def ohs(s):
                    q, r = divmod(s * 128, csz)
                    return ohc[q][:, r:r + 128]

                # conv: paired PSUM tiles [128, 2, 512]; z_t lives at
                # pair t//2, half t%2 (bank-aligned), cols 0:300
                zp = [None] * NP

                def zt(t):
                    return zp[t // 2][:, t % 2, 0:HIDDEN]

                # two independent max chains (A: even pairs, B: odd pairs)
                # so dependent DVE ops from the two chains interleave and
                # hide each other's semaphore latency.
                rmA = rmpool.tile([128, 2, HIDDEN], FP32, tag="rm", name="rmA")
                rmB = rmpool.tile([128, 2, HIDDEN], FP32, tag="rm", name="rmB")
                for s in range(L):
                    oh_s = ohs(s)
                    if s == 0:
                        zp[0] = pz.tile([128, 2, 512], FP32, tag="zp",
                                        name="zp0")
                    if s + 1 < L:
                        if (s + 1) % 2 == 0:
                            zp[(s + 1) // 2] = pz.tile([128, 2, 512], FP32,
                                                       tag="zp", name="zpn")
                        nc.tensor.matmul(zt(s + 1), oh_s, wfuse[:, 0, :],
                                         start=True, stop=False)
                    nc.tensor.matmul(zt(s), oh_s, wfuse[:, 1, :],
                                     start=(s == 0), stop=(s == L - 1))
                    if s >= 1:
                        nc.tensor.matmul(zt(s - 1), oh_s, wfuse[:, 2, :],
                                         start=False, stop=True)
                    # pair j (t=2j,2j+1) completes after the MMs emitted at
                    # s == 2j+2 (its k=2 tap lands at s=2j+2).
                    if s >= 2 and s % 2 == 0:
                        j = (s - 2) // 2
                        rm = rmA if j % 2 == 0 else rmB
                        if j <= 1:
                            nc.scalar.copy(rm[:, :, :], zp[j][:, :, 0:HIDDEN])
                        else:
                            nc.vector.tensor_tensor(
                                rm[:, :, :], zp[j][:, :, 0:HIDDEN],
                                rm[:, :, :], Max)
                        if j == 6:
                            # chains A (0,2,4,6) and B (1,3,5) both done:
                            # merge them now, during the last matmuls, so
                            # only pair 7 + the fold trail the PE.
                            nc.vector.tensor_tensor(rmA[:, :, :],
                                                    rmB[:, :, :],
                                                    rmA[:, :, :], Max)
                nc.vector.tensor_tensor(rmA[:, :, :],
                                        zp[NP - 1][:, :, 0:HIDDEN],
                                        rmA[:, :, :], Max)

                # relu(max(a,b)) == max(max(a,0),b): one fused DVE op
                # (bias is pre-folded into the middle-tap table on host)
                outt = outpool.tile([128, HIDDEN], FP32, tag="outt")
                nc.vector.scalar_tensor_tensor(
                    outt[:], rmA[:, 0, :], 0.0, rmA[:, 1, :], Max, Max)
                nc.sync.dma_start(out_d.ap()[g * 128:(g + 1) * 128, :],
                                  outt[:])

    nc.compile()
    _cache[key] = nc
    return nc


def _prep_inputs(words_batch, emb_table, conv_w, conv_b):
    emb = np.asarray(emb_table, dtype=np.float32)
    w = np.asarray(conv_w, dtype=np.float32)
    b = np.asarray(conv_b, dtype=np.float32)
    words = np.asarray(words_batch)

    wfuse = np.stack([emb @ w[:, :, k].T for k in range(KSIZE)], axis=0)
    # tap k=1 fires exactly once for every valid t, so the conv bias can be
    # folded into the middle-tap table: z_t picks it up exactly once.
    wfuse[1] += b[None, :]
    wfuse = np.ascontiguousarray(wfuse, dtype=np.float32)
    iotaf = np.arange(128, dtype=np.float32).reshape(128, 1)

    in_maps = []
    for c in range(NCORES):
        wc = words[:, c * NW:(c + 1) * NW].astype(np.float32)   # [16, 2048]
        wg = np.ascontiguousarray(
            wc.reshape(L, GROUPS, 128).transpose(1, 0, 2)
        ).reshape(GROUPS, L * 128)
        in_maps.append({"wordsg": wg, "wfuse": wfuse, "iotaf": iotaf})
    return in_maps


def _run(in_maps, iters=1):
    nc = _build(iters)
    return run_bass_kernel_spmd(nc, in_maps, list(range(NCORES)),
                                trace=False)


def kernel(words_batch, emb_table, conv_w, conv_b):
    in_maps = _prep_inputs(words_batch, emb_table, conv_w, conv_b)
    res = _run(in_maps, iters=1)
    out = np.concatenate([res.results[c]["out"] for c in range(NCORES)],
                         axis=0)
    return out



# revision 5
# speedup vs baseline: 1.1853x; 1.1853x over previous
"""ConvNetWordEncoder Trainium2 kernel.

Computes, for a batch of words (each a sequence of L=16 character ids):
  x = emb_table[words]                          # [L, N, D] character embeddings
  y = conv1d(x, conv_w, pad=1) + b              # [N, D, L]
  out = max_t relu(y)                           # [N, D]

Algebraic core: the embedding lookup is linear, so the conv collapses into
per-tap fused tables M_k = E @ W_k^T [128 letters, 300 ch]:
  z_t = sum_k M_k^T @ onehot(letter at t+k-1)
Bias is folded into the middle tap (fires exactly once per t); relu commutes
with the temporal max.

This version targets the fp8 DoubleRow path of the PE: each fused table is
split on the host into hi = fp8(M) and lo = fp8(M - hi).  One DoubleRow
matmul computes oh^T @ hi + oh^T @ lo = oh^T @ (hi + lo) - the lo term
cancels the fp8 quantization, so per-tap error is ~1e-3 while the matmul
runs at 0.5 cycles/row (2x over fp32r).  The one-hot (exact in fp8: 0/1)
is built on the host and DMA'd in, so no engine builds it on-chip; the
same one-hot block feeds both DR slots via a stride-0 broadcast.

Per core (2048 words, 16 groups of 128), per group:
  - PE: 46 DoubleRow matmuls accumulate z_t [128w, 300c] fp32 in PSUM,
    two z per [128, 2, 512] tile (bank-aligned halves), 4 tiles cycling.
  - Drain (one PSUM operand per op; gpsimd cannot touch PSUM):
      ACT seeds 4 chains:  rm_i = relu(pair_{2i})      (PSUM -> SBUF bf16)
      DVE chains:          rm_i = max(max(pair_{2i+1}, 0), rm_i)  (fused stt)
  - Pool merges the 4 bf16 chains in SBUF; DVE folds the final pair's
    halves.  Output is stored bf16 and widened to fp32 on the host.
Drains are interleaved (ACT takes even pairs, DVE odd) so PSUM tiles free
before the PE needs them back 4 pairs later.
"""

import numpy as np
import ml_dtypes
from contextlib import ExitStack

import concourse.tile as tile
from concourse import bacc, mybir
from concourse.bass_utils import run_bass_kernel_spmd

HIDDEN = 300
NLET = 128
KSIZE = 3
L = 16
NWORDS = 16384
NCORES = 8
NW = NWORDS // NCORES          # 2048 words per core
GROUPS = NW // 128             # 16 groups of 128 words
COLS = L * 128                 # one-hot columns per group
FP32 = mybir.dt.float32
BF16 = mybir.dt.bfloat16
FP8 = mybir.dt.float8e4
DR = mybir.MatmulPerfMode.DoubleRow

_cache = {}


def _build(iters=1):
    key = iters
    if key in _cache:
        return _cache[key]
    nc = bacc.Bacc("TRN2", target_bir_lowering=False, debug=False,
                   num_devices=NCORES)

    oh_d = nc.dram_tensor("oh", [GROUPS, NLET, COLS], FP8,
                          kind="ExternalInput")
    wf_d = nc.dram_tensor("wf", [NLET, KSIZE, 2, HIDDEN], FP8,
                          kind="ExternalInput")
    out_d = nc.dram_tensor("out", [NW, HIDDEN], BF16, kind="ExternalOutput")

    Relu = mybir.ActivationFunctionType.Relu
    Max = mybir.AluOpType.max

    with tile.TileContext(nc) as tc, ExitStack() as ctx:
        const = ctx.enter_context(tc.tile_pool(name="const", bufs=1))
        ohpool = ctx.enter_context(tc.tile_pool(name="ohp", bufs=3))
        rmpool = ctx.enter_context(tc.tile_pool(name="rm", bufs=10))
        outpool = ctx.enter_context(tc.tile_pool(name="outp", bufs=4))
        pz = ctx.enter_context(tc.tile_pool(name="pz", bufs=4, space="PSUM"))

        wf = const.tile([NLET, KSIZE, 2, HIDDEN], FP8)
        nc.sync.dma_start(wf[:], wf_d.ap()[:])

        # PE p-state pre-warm: dummy DoubleRow matmuls dependent only on the
        # (tiny, fast) table DMA keep the PE busy while the first one-hot
        # loads, so real matmuls enter closer to full clock.
        warm_ps = pz.tile([128, 2, 512], FP32, tag="zp", name="warm")
        for _ in range(24):
            nc.tensor.matmul(warm_ps[:, 0, 0:HIDDEN],
                             wf[:, 0, 0, 0:128].unsqueeze(1)
                             .broadcast_to([NLET, 2, 128]),
                             wf[:, 0, :, :], start=True, stop=True,
                             perf_mode=DR)

        NP = L // 2  # 8 z-pairs per group
        for it in range(iters):
            for g in range(GROUPS):
                oh = ohpool.tile([NLET, COLS], FP8, tag="oh", name="oh")
                if it == 0 and g == 0:
                    # first group in 4 chunks: first matmuls gate on 1/4
                    csz = COLS // 4
                    for q in range(4):
                        nc.sync.dma_start(oh[:, q * csz:(q + 1) * csz],
                                          oh_d.ap()[g, :, q * csz:(q + 1) * csz])
                else:
                    nc.sync.dma_start(oh[:], oh_d.ap()[g])

                def ohs(s):
                    blk = oh[:, s * 128:(s + 1) * 128]
                    return blk.unsqueeze(1).broadcast_to([NLET, 2, 128])

                zp = [None] * NP

                def zt(t):
                    return zp[t // 2][:, t % 2, 0:HIDDEN]

                # drain plan (walrus rules: one PSUM operand per op, gpsimd
                # cannot touch PSUM, no tensor_tensor on gpsimd):
                #   ACT relu-seeds pairs 0,2,4,6,7 -> rm0..rm4 (bf16 SBUF)
                #   DVE stt-drains pairs 1,3,5 onto rm0,rm1,rm2 (relu fused)
                #   DVE merges the 5 chains + half-fold, all bf16 2x mode
                ACT_SEED = {0: 0, 2: 1, 4: 2, 6: 3, 7: 4}
                DVE_CHAIN = {1: 0, 3: 1, 5: 2}
                rm = [None] * 5

                def drain(j):
                    pair = zp[j][:, :, 0:HIDDEN]
                    if j in ACT_SEED:
                        r = rmpool.tile([128, 2, HIDDEN], BF16, tag="rm",
                                        name="rmseed")
                        nc.scalar.activation(r[:], pair, Relu)
                        rm[ACT_SEED[j]] = r
                    else:
                        r = rm[DVE_CHAIN[j]]
                        nc.vector.scalar_tensor_tensor(
                            r[:], pair, 0.0, r[:], Max, Max)

                for s in range(L):
                    oh_s = ohs(s)
                    if s == 0:
                        zp[0] = pz.tile([128, 2, 512], FP32, tag="zp",
                                        name="zp0")
                    if s + 1 < L:
                        if (s + 1) % 2 == 0:
                            zp[(s + 1) // 2] = pz.tile([128, 2, 512], FP32,
                                                       tag="zp", name="zpn")
                        nc.tensor.matmul(zt(s + 1), oh_s, wf[:, 0, :, :],
                                         start=True, stop=False, perf_mode=DR)
                    nc.tensor.matmul(zt(s), oh_s, wf[:, 1, :, :],
                                     start=(s == 0), stop=(s == L - 1),
                                     perf_mode=DR)
                    if s >= 1:
                        nc.tensor.matmul(zt(s - 1), oh_s, wf[:, 2, :, :],
                                         start=False, stop=True, perf_mode=DR)
                    # pair j (z_2j, z_2j+1) completes with the tap-2 matmul
                    # emitted at s == 2j+2 (pair 7 at s == 15).
                    if s >= 2 and s % 2 == 0:
                        drain((s - 2) // 2)
                drain(6)
                drain(7)

                # bf16 merge tree on DVE (rm4 = latest pair merges last)
                nc.vector.tensor_tensor(rm[0][:], rm[0][:], rm[1][:], Max)
                nc.vector.tensor_tensor(rm[2][:], rm[2][:], rm[3][:], Max)
                nc.vector.tensor_tensor(rm[0][:], rm[0][:], rm[2][:], Max)
                nc.vector.tensor_tensor(rm[0][:], rm[0][:], rm[4][:], Max)
                outt = outpool.tile([128, HIDDEN], BF16, tag="outt")
                nc.vector.tensor_tensor(outt[:], rm[0][:, 0, :],
                                        rm[0][:, 1, :], Max)
                nc.sync.dma_start(out_d.ap()[g * 128:(g + 1) * 128, :],
                                  outt[:])

    nc.compile()
    _cache[key] = nc
    return nc


def _prep_inputs(words_batch, emb_table, conv_w, conv_b):
    emb = np.asarray(emb_table, dtype=np.float32)
    w = np.asarray(conv_w, dtype=np.float32)
    b = np.asarray(conv_b, dtype=np.float32)
    words = np.asarray(words_batch)

    # fused per-tap tables [3, 128, 300]; bias folded into the middle tap
    wfuse = np.stack([emb @ w[:, :, k].T for k in range(KSIZE)], axis=0)
    wfuse[1] += b[None, :]
    # hi/lo fp8 split: hi + lo reconstructs wfuse to ~1e-3
    hi = wfuse.astype(ml_dtypes.float8_e4m3)
    lo = (wfuse - hi.astype(np.float32)).astype(ml_dtypes.float8_e4m3)
    # [letters, tap, {hi,lo}, ch]
    wf = np.stack([np.asarray(hi), np.asarray(lo)], axis=2).transpose(1, 0, 2, 3)
    wf = np.ascontiguousarray(wf)

    ar = np.arange(NLET, dtype=words.dtype)
    in_maps = []
    for c in range(NCORES):
        wc = np.asarray(words[:, c * NW:(c + 1) * NW])       # [16, 2048]
        wg = wc.reshape(L, GROUPS, 128).transpose(1, 0, 2)   # [g, t, wi]
        # one-hot [g, letter, t*128+wi], exact 0/1 in fp8
        oh = (wg[:, None, :, :] == ar[None, :, None, None])
        oh = oh.reshape(GROUPS, NLET, COLS).astype(ml_dtypes.float8_e4m3)
        in_maps.append({"oh": oh, "wf": wf})
    return in_maps


def _run(in_maps, iters=1):
    nc = _build(iters)
    return run_bass_kernel_spmd(nc, in_maps, list(range(NCORES)),
                                trace=False)


def kernel(words_batch, emb_table, conv_w, conv_b):
    in_maps = _prep_inputs(words_batch, emb_table, conv_w, conv_b)
    res = _run(in_maps, iters=1)
    out = np.concatenate(
        [np.asarray(res.results[c]["out"]).astype(np.float32)
         for c in range(NCORES)], axis=0)
    return out


# revision 9
# speedup vs baseline: 1.4437x; 1.2179x over previous
"""ConvNetWordEncoder Trainium2 kernel.

Computes, for a batch of words (each a sequence of L=16 character ids):
  x = emb_table[words]                          # [L, N, D] character embeddings
  y = conv1d(x, conv_w, pad=1) + b              # [N, D, L]
  out = max_t relu(y)                           # [N, D]

Algebraic core: the embedding lookup is linear, so the conv collapses into
per-tap fused tables M_k = E @ W_k^T [128 letters, 300 ch]:
  z_t = sum_k M_k^T @ onehot(letter at t+k-1)
Bias is folded into the middle tap (fires exactly once per t); relu commutes
with the temporal max.

This version targets the fp8 DoubleRow path of the PE: each fused table is
split on the host into hi = fp8(M) and lo = fp8(M - hi).  One DoubleRow
matmul computes oh^T @ hi + oh^T @ lo = oh^T @ (hi + lo) - the lo term
cancels the fp8 quantization, so per-tap error is ~1e-3 while the matmul
runs at 0.5 cycles/row (2x over fp32r).  The one-hot (exact in fp8: 0/1)
is built on the host and DMA'd in, so no engine builds it on-chip; the
same one-hot block feeds both DR slots via a stride-0 broadcast.

Per core (2048 words, 16 groups of 128), per group:
  - PE: 46 DoubleRow matmuls accumulate z_t [128w, 300c] fp32 in PSUM,
    two z per [128, 2, 512] tile (bank-aligned halves), 4 tiles cycling.
  - Drain (one PSUM operand per op; gpsimd cannot touch PSUM):
      ACT seeds 4 chains:  rm_i = relu(pair_{2i})      (PSUM -> SBUF bf16)
      DVE chains:          rm_i = max(max(pair_{2i+1}, 0), rm_i)  (fused stt)
  - Pool merges the 4 bf16 chains in SBUF; DVE folds the final pair's
    halves.  Output is stored bf16 and widened to fp32 on the host.
Drains are interleaved (ACT takes even pairs, DVE odd) so PSUM tiles free
before the PE needs them back 4 pairs later.
"""

import numpy as np
import ml_dtypes
from contextlib import ExitStack

import concourse.tile as tile
from concourse import bacc, mybir
from concourse.bass_utils import run_bass_kernel_spmd

HIDDEN = 300
NLET = 128
KSIZE = 3
L = 16
NWORDS = 16384
NCORES = 8
NW = NWORDS // NCORES          # 2048 words per core
GROUPS = NW // 128             # 16 groups of 128 words
COLS = L * 128                 # one-hot columns per group
FP32 = mybir.dt.float32
BF16 = mybir.dt.bfloat16
FP8 = mybir.dt.float8e4
DR = mybir.MatmulPerfMode.DoubleRow

_cache = {}


def _build(iters=1):
    key = iters
    if key in _cache:
        return _cache[key]
    nc = bacc.Bacc("TRN2", target_bir_lowering=False, debug=False,
                   num_devices=NCORES)

    oh_d = nc.dram_tensor("oh", [GROUPS, NLET, COLS], FP8,
                          kind="ExternalInput")
    wf_d = nc.dram_tensor("wf", [NLET, KSIZE, 2, HIDDEN], FP8,
                          kind="ExternalInput")
    out_d = nc.dram_tensor("out", [NW, HIDDEN], BF16, kind="ExternalOutput")

    Relu = mybir.ActivationFunctionType.Relu
    Max = mybir.AluOpType.max

    with tile.TileContext(nc) as tc, ExitStack() as ctx:
        const = ctx.enter_context(tc.tile_pool(name="const", bufs=1))
        ohpool = ctx.enter_context(tc.tile_pool(name="ohp", bufs=3))
        rmpool = ctx.enter_context(tc.tile_pool(name="rm", bufs=10))
        outpool = ctx.enter_context(tc.tile_pool(name="outp", bufs=4))
        pz = ctx.enter_context(tc.tile_pool(name="pz", bufs=4, space="PSUM"))

        wf = const.tile([NLET, KSIZE, 2, HIDDEN], FP8)
        nc.sync.dma_start(wf[:], wf_d.ap()[:])

        # PE p-state pre-warm: dummy DoubleRow matmuls on a memset tile (no
        # DMA dependency - starts almost immediately) keep the PE busy until
        # the first one-hot lands, so real matmuls enter close to full clock.
        wz = const.tile([NLET, 2, HIDDEN], FP8)
        nc.vector.memset(wz[:], 0.0)
        warm_ps = pz.tile([128, 2, 512], FP32, tag="zp", name="warm")
        for _ in range(18):
            nc.tensor.matmul(warm_ps[:, 0, 0:HIDDEN],
                             wz[:, 0, 0:128].unsqueeze(1)
                             .broadcast_to([NLET, 2, 128]),
                             wz[:], start=True, stop=True,
                             perf_mode=DR)

        NP = L // 2  # 8 z-pairs per group
        for it in range(iters):
            for g in range(GROUPS):
                oh = ohpool.tile([NLET, COLS], FP8, tag="oh", name="oh")
                if it == 0 and g == 0:
                    # first group in 4 chunks: first matmuls gate on 1/4
                    csz = COLS // 4
                    for q in range(4):
                        nc.sync.dma_start(oh[:, q * csz:(q + 1) * csz],
                                          oh_d.ap()[g, :, q * csz:(q + 1) * csz])
                else:
                    nc.sync.dma_start(oh[:], oh_d.ap()[g])

                def ohs(s):
                    blk = oh[:, s * 128:(s + 1) * 128]
                    return blk.unsqueeze(1).broadcast_to([NLET, 2, 128])

                zp = [None] * NP

                def zt(t):
                    return zp[t // 2][:, t % 2, 0:HIDDEN]

                # drain plan (walrus rules: one PSUM operand per op, gpsimd
                # cannot touch PSUM, no tensor_tensor on gpsimd):
                #   ACT relu-seeds 5 or 6 pairs -> rm chains (bf16 SBUF)
                #   DVE stt-drains the rest onto chains (relu fused)
                #   DVE merges the chains + half-fold, all bf16 2x mode
                # a alternates 5/6 by group parity to balance ACT vs DVE.
                if g % 2 == 0:
                    ACT_SEED = {0: 0, 2: 1, 4: 2, 6: 3, 7: 4}
                    DVE_CHAIN = {1: 0, 3: 1, 5: 2}
                else:
                    ACT_SEED = {0: 0, 2: 1, 4: 2, 5: 3, 6: 4, 7: 5}
                    DVE_CHAIN = {1: 0, 3: 1}
                rm = [None] * 6

                def drain(j):
                    pair = zp[j][:, :, 0:HIDDEN]
                    if j in ACT_SEED:
                        r = rmpool.tile([128, 2, HIDDEN], BF16, tag="rm",
                                        name="rmseed")
                        nc.scalar.activation(r[:], pair, Relu)
                        rm[ACT_SEED[j]] = r
                    else:
                        r = rm[DVE_CHAIN[j]]
                        nc.vector.scalar_tensor_tensor(
                            r[:], pair, 0.0, r[:], Max, Max)

                for s in range(L):
                    oh_s = ohs(s)
                    if s == 0:
                        zp[0] = pz.tile([128, 2, 512], FP32, tag="zp",
                                        name="zp0")
                    if s + 1 < L:
                        if (s + 1) % 2 == 0:
                            zp[(s + 1) // 2] = pz.tile([128, 2, 512], FP32,
                                                       tag="zp", name="zpn")
                        nc.tensor.matmul(zt(s + 1), oh_s, wf[:, 0, :, :],
                                         start=True, stop=False, perf_mode=DR)
                    nc.tensor.matmul(zt(s), oh_s, wf[:, 1, :, :],
                                     start=(s == 0), stop=(s == L - 1),
                                     perf_mode=DR)
                    if s >= 1:
                        nc.tensor.matmul(zt(s - 1), oh_s, wf[:, 2, :, :],
                                         start=False, stop=True, perf_mode=DR)
                    # pair j (z_2j, z_2j+1) completes with the tap-2 matmul
                    # emitted at s == 2j+2 (pair 7 at s == 15).
                    if s >= 2 and s % 2 == 0:
                        drain((s - 2) // 2)
                drain(7)

                # bf16 merge tree on DVE (latest pairs merge last)
                nc.vector.tensor_tensor(rm[0][:], rm[0][:], rm[1][:], Max)
                nc.vector.tensor_tensor(rm[2][:], rm[2][:], rm[3][:], Max)
                nc.vector.tensor_tensor(rm[0][:], rm[0][:], rm[2][:], Max)
                if g % 2 == 0:
                    nc.vector.tensor_tensor(rm[0][:], rm[0][:], rm[4][:], Max)
                else:
                    nc.vector.tensor_tensor(rm[4][:], rm[4][:], rm[5][:], Max)
                    nc.vector.tensor_tensor(rm[0][:], rm[0][:], rm[4][:], Max)
                outt = outpool.tile([128, HIDDEN], BF16, tag="outt")
                nc.vector.tensor_tensor(outt[:], rm[0][:, 0, :],
                                        rm[0][:, 1, :], Max)
                nc.sync.dma_start(out_d.ap()[g * 128:(g + 1) * 128, :],
                                  outt[:])

    nc.compile()
    _cache[key] = nc
    return nc


def _prep_inputs(words_batch, emb_table, conv_w, conv_b):
    emb = np.asarray(emb_table, dtype=np.float32)
    w = np.asarray(conv_w, dtype=np.float32)
    b = np.asarray(conv_b, dtype=np.float32)
    words = np.asarray(words_batch)

    # fused per-tap tables [3, 128, 300]; bias folded into the middle tap
    wfuse = np.stack([emb @ w[:, :, k].T for k in range(KSIZE)], axis=0)
    wfuse[1] += b[None, :]
    # hi/lo fp8 split: hi + lo reconstructs wfuse to ~1e-3
    hi = wfuse.astype(ml_dtypes.float8_e4m3)
    lo = (wfuse - hi.astype(np.float32)).astype(ml_dtypes.float8_e4m3)
    # [letters, tap, {hi,lo}, ch]
    wf = np.stack([np.asarray(hi), np.asarray(lo)], axis=2).transpose(1, 0, 2, 3)
    wf = np.ascontiguousarray(wf)

    ar = np.arange(NLET, dtype=words.dtype)
    in_maps = []
    for c in range(NCORES):
        wc = np.asarray(words[:, c * NW:(c + 1) * NW])       # [16, 2048]
        wg = wc.reshape(L, GROUPS, 128).transpose(1, 0, 2)   # [g, t, wi]
        # one-hot [g, letter, t*128+wi], exact 0/1 in fp8
        oh = (wg[:, None, :, :] == ar[None, :, None, None])
        oh = oh.reshape(GROUPS, NLET, COLS).astype(ml_dtypes.float8_e4m3)
        in_maps.append({"oh": oh, "wf": wf})
    return in_maps


def _run(in_maps, iters=1):
    nc = _build(iters)
    return run_bass_kernel_spmd(nc, in_maps, list(range(NCORES)),
                                trace=False)


def kernel(words_batch, emb_table, conv_w, conv_b):
    in_maps = _prep_inputs(words_batch, emb_table, conv_w, conv_b)
    res = _run(in_maps, iters=1)
    out = np.concatenate(
        [np.asarray(res.results[c]["out"]).astype(np.float32)
         for c in range(NCORES)], axis=0)
    return out
